# revision 16
# baseline (speedup 1.0000x reference)
"""Multi-head causal attention on 8 Trainium2 NeuronCores.

Sharding: tensor-parallel over heads (4 heads/core) for QKV + attention;
AllGather of per-core context chunks (bf16, 4MB/core); o_proj column-sharded
(each core computes out[:, c*512:(c+1)*512] for all tokens). Final bias bo
and the output concatenation happen on host.

Compute dtype: bf16 operands, fp32 PSUM accumulation and softmax statistics.
Layouts: projections produce q/k/v feature-major ([feat, token]) so the
k-dim contraction runs on the partition axis everywhere; P tiles are
PE-transposed for the PV matmul; x is DMA-transposed on load (bf16 XBAR).
"""

import numpy as np
import ml_dtypes

import concourse.bass as bass
import concourse.mybir as mybir
import concourse.tile as tile
from concourse.bass_utils import run_bass_kernel_spmd
from concourse.masks import make_identity, make_causal_mask
from bass_rust import ScopedClock
from concourse.bass import _add_dep_helper

B, L, D = 2, 2048, 4096
H, HD = 32, 128
N_CORES = 8
HPC = H // N_CORES          # heads per core = 4
FPC = HPC * HD              # features per core = 512
T = B * L                   # tokens = 4096
SCALE = HD ** -0.5
MB = 512                    # stage-1 token block
BF16 = mybir.dt.bfloat16
F32 = mybir.dt.float32
AF = mybir.ActivationFunctionType
OP = mybir.AluOpType


class SplitDrainTileContext(tile.TileContext):
    """Tail drain in this walrus build allows only a couple of sync waits per
    instruction; split the global-clock waits across multiple drains."""

    MAX_WAITS = 1

    def _drain_and_barrier(self, tick_clock, wait_clock):
        nc = self.nc
        drain_inst = nc.sync.drain()
        wait_clock.add_sem_waits(
            drain_inst.ins, ScopedClock({None: tick_clock.global_clock})
        )
        si = drain_inst.ins.sync_info
        waits = list(si.on_wait) if si is not None and si.on_wait else []
        mw = self.MAX_WAITS
        if len(waits) > mw:
            si.on_wait = waits[:mw]
            rest = waits[mw:]
            for i in range(0, len(rest), mw):
                extra = nc.sync.drain()
                extra.ins.sync_info = mybir.SyncInfo(
                    on_wait=rest[i:i + mw], on_update=[]
                )
        nc.all_engine_barrier()
        assert self.sems is not None
        popped = nc._tile_sem_poison_stack.pop()
        assert popped is self._sem_poison
        nc.clear_and_free_semaphores(list(self.sems.allocated().values()))
        nc.all_engine_barrier()


def split_excess_waits(nc, max_waits=1):
    """walrus in this container rejects instructions with more than a couple of
    sync-wait commands. Split excess waits onto NoOps inserted just before the
    instruction on the same engine (same-engine program order preserved)."""
    f = nc.main_func
    new_blocks = []
    n = 0
    for bb in f.blocks:
        out = []
        changed = False
        for ins in bb.instructions:
            si = ins.sync_info
            waits = list(si.on_wait) if si is not None and si.on_wait else []
            if len(waits) > max_waits:
                keep, rest = waits[-max_waits:], waits[:-max_waits]
                for i in range(0, len(rest), max_waits):
                    nop = mybir.InstNoOp(name=f"I-wsplit-{n}", engine=ins.engine)
                    nop.sync_info = mybir.SyncInfo(
                        on_wait=rest[i:i + max_waits], on_update=[])
                    out.append(nop)
                    n += 1
                si.on_wait = keep
                changed = True
            out.append(ins)
        if changed:
            nb = mybir.BasicBlock(name=bb.name, instructions=out)
            for attr in ("IsExit", "IsLoopEntry", "IsPredicated"):
                v = getattr(bb, attr)
                if v is not None:
                    setattr(nb, attr, v)
            new_blocks.append(nb)
        else:
            new_blocks.append(bb)
    f.blocks = new_blocks
    return n


def build_program():
    nc = bass.Bass("TRN2", target_bir_lowering=False, debug=False,
                   num_devices=N_CORES)

    x_sh = nc.dram_tensor("x_sh", [T // N_CORES, D], BF16, kind="ExternalInput")
    wq = nc.dram_tensor("wq", [D, FPC], BF16, kind="ExternalInput")
    wk = nc.dram_tensor("wk", [D, FPC], BF16, kind="ExternalInput")
    wv = nc.dram_tensor("wv", [D, FPC], BF16, kind="ExternalInput")
    wo = nc.dram_tensor("wo", [D, FPC], BF16, kind="ExternalInput")
    bq = nc.dram_tensor("bq", [FPC], F32, kind="ExternalInput")
    bk = nc.dram_tensor("bk", [FPC], F32, kind="ExternalInput")
    bv = nc.dram_tensor("bv", [FPC], F32, kind="ExternalInput")

    k_out = nc.dram_tensor("k_out", [B, HPC, L, HD], BF16, kind="ExternalOutput")
    v_out = nc.dram_tensor("v_out", [B, HPC, L, HD], BF16, kind="ExternalOutput")
    out_cols = nc.dram_tensor("out_cols", [T, FPC], F32, kind="ExternalOutput")
    import os
    DBG = os.environ.get("KDBG") == "1"
    if DBG:
        dbg_xt = nc.dram_tensor("dbg_xt", [128, MB], BF16, kind="ExternalOutput")
        dbg_qt = nc.dram_tensor("dbg_qt", [FPC, T], BF16, kind="ExternalOutput")

    KT = D // 128            # 32 k-tiles
    MBLKS = T // MB          # stage-1 m blocks
    ITILES = L // 128        # 16 i-tiles per batch
    IBLKS = L // 512         # 4 i-blocks per batch

    with SplitDrainTileContext(nc) as tc:
        with tc.tile_pool(name="dram", bufs=1, space="DRAM") as dram:
            ctx_local = [dram.tile([FPC, L], BF16, name=f"ctx_local{b}")
                         for b in range(B)]
            ctx_all = [dram.tile([N_CORES, FPC, L], BF16, addr_space="Shared",
                                 name=f"ctx_all{b}") for b in range(B)]
            x_loc = dram.tile([T // N_CORES, D], BF16)
            x_all = dram.tile([N_CORES, T // N_CORES, D], BF16, addr_space="Shared")
            x_loc_wr = nc.sync.dma_start(x_loc[:], x_sh.ap()[:])
            x_ag = nc.gpsimd.collective_compute(
                "AllGather", OP.bypass,
                replica_groups=[list(range(N_CORES))],
                ins=[x_loc.opt()],
                outs=[x_all.opt()],
            )
            _add_dep_helper(x_ag.ins, x_loc_wr.ins, sync=True, reason="xag after xloc write")
            qT_d = dram.tile([FPC, T], BF16)
            kT_d = dram.tile([FPC, T], BF16)
            vT_d = dram.tile([FPC, T], BF16)

            if True:
                # ---------------- stage 1: QKV projections ----------------
                with tc.tile_pool(name="wres", bufs=1) as wres, \
                     tc.tile_pool(name="xt", bufs=2) as xtp, \
                     tc.tile_pool(name="stg1", bufs=4) as stg1, \
                     tc.tile_pool(name="ps1", bufs=4, space="PSUM") as ps1:
                    wq_sb = wres.tile([128, KT, FPC], BF16)
                    wk_sb = wres.tile([128, KT, FPC], BF16)
                    wv_sb = wres.tile([128, KT, FPC], BF16)
                    nc.sync.dma_start(wq_sb[:], wq.ap().rearrange("(ko p) n -> p ko n", p=128))
                    nc.sync.dma_start(wk_sb[:], wk.ap().rearrange("(ko p) n -> p ko n", p=128))
                    nc.sync.dma_start(wv_sb[:], wv.ap().rearrange("(ko p) n -> p ko n", p=128))
                    bq_sb = wres.tile([128, HPC], F32)
                    bk_sb = wres.tile([128, HPC], F32)
                    bv_sb = wres.tile([128, HPC], F32)
                    nc.sync.dma_start(bq_sb[:], bq.ap().rearrange("(o p) -> p o", p=128))
                    nc.sync.dma_start(bk_sb[:], bk.ap().rearrange("(o p) -> p o", p=128))
                    nc.sync.dma_start(bv_sb[:], bv.ap().rearrange("(o p) -> p o", p=128))

                    projs = [(wq_sb, bq_sb, qT_d), (wk_sb, bk_sb, kT_d), (wv_sb, bv_sb, vT_d)]
                    qkv_wr = {}
                    for mb in range(MBLKS):
                        xT = xtp.tile([128, KT, MB], BF16, tag="xT")
                        for kt in range(KT):
                            xt_tr = nc.sync.dma_start_transpose(
                                xT[:, kt, :],
                                x_all[mb, :, kt * 128:(kt + 1) * 128],
                            )
                            _add_dep_helper(xt_tr.ins, x_ag.ins, sync=True,
                                            reason="xT after x allgather")
                        if DBG and mb == 3:
                            nc.sync.dma_start(dbg_xt.ap()[:], xT[:, 2, :])
                        for pi, (w_sb, b_sb, oT) in enumerate(projs):
                            for nt in range(HPC):
                                ps = ps1.tile([128, MB], F32, tag="ps1")
                                for kt in range(KT):
                                    nc.tensor.matmul(
                                        ps, w_sb[:, kt, nt * 128:(nt + 1) * 128],
                                        xT[:, kt, :],
                                        start=(kt == 0), stop=(kt == KT - 1),
                                    )
                                stg = stg1.tile([128, MB], BF16, tag="stg1")
                                nc.scalar.activation(
                                    stg, ps,
                                    AF.Identity, bias=b_sb[:, nt:nt + 1],
                                )
                                qkv_wr[(pi, nt, mb)] = nc.sync.dma_start(
                                    oT[nt * 128:(nt + 1) * 128, mb * MB:(mb + 1) * MB],
                                    stg)

                if DBG:
                    nc.sync.dma_start(dbg_qt.ap()[:], qT_d[:])
                # ---------------- stage 2: attention ----------------
                with tc.tile_pool(name="const2", bufs=1) as cpool, \
                     tc.tile_pool(name="qkvh", bufs=3) as qkvhp, \
                     tc.tile_pool(name="kstg", bufs=4) as kstgp, \
                     tc.tile_pool(name="vtok", bufs=2) as vtokp, \
                     tc.tile_pool(name="prow", bufs=3) as prowp, \
                     tc.tile_pool(name="ptsl", bufs=2) as ptp, \
                     tc.tile_pool(name="stat", bufs=8) as statp, \
                     tc.tile_pool(name="cstg", bufs=4) as cstgp, \
                     tc.tile_pool(name="pstr", bufs=3, space="PSUM") as pstr, \
                     tc.tile_pool(name="pss", bufs=3, space="PSUM") as pss, \
                     tc.tile_pool(name="psct", bufs=2, space="PSUM") as psct:
                    ctx_wr = [[], []]
                    ident = cpool.tile([128, 128], BF16)
                    make_identity(nc, ident)
                    cmask = cpool.tile([128, 128], F32)
                    make_causal_mask(nc, cmask, mask_val=-1e10)

                    for b in range(B):
                        for h in range(HPC):
                            t0 = b * L
                            rows = slice(h * 128, (h + 1) * 128)
                            cols = slice(t0, t0 + L)
                            qh = qkvhp.tile([128, L], BF16, tag="qh")
                            kh = qkvhp.tile([128, L], BF16, tag="kh")
                            vh = qkvhp.tile([128, L], BF16, tag="vh")
                            mbs_b = range(4 * b, 4 * b + 4)
                            for pi, (ld_t, src_t) in enumerate(
                                    [(qh, qT_d), (kh, kT_d), (vh, vT_d)]):
                                ld = nc.sync.dma_start(ld_t[:], src_t[rows, cols])
                                for mb_ in mbs_b:
                                    _add_dep_helper(
                                        ld.ins, qkv_wr[(pi, h, mb_)].ins,
                                        sync=True, reason="qkv load after proj write")
                            # token-major v (matmul operand + output), k (output)
                            v_sb = vtokp.tile([128, ITILES, HD], BF16, tag="v_sb")
                            for jt in range(ITILES):
                                ptv = pstr.tile([128, 128], BF16, tag="ptr")
                                nc.tensor.transpose(
                                    ptv, vh[:, jt * 128:(jt + 1) * 128], ident)
                                nc.any.tensor_copy(v_sb[:, jt, :], ptv)
                                nc.sync.dma_start(
                                    v_out.ap()[b, h, jt * 128:(jt + 1) * 128, :],
                                    v_sb[:, jt, :])
                                ptk = pstr.tile([128, 128], BF16, tag="ptr")
                                nc.tensor.transpose(
                                    ptk, kh[:, jt * 128:(jt + 1) * 128], ident)
                                kstg = kstgp.tile([128, 128], BF16, tag="kstg")
                                nc.any.tensor_copy(kstg, ptk)
                                nc.sync.dma_start(
                                    k_out.ap()[b, h, jt * 128:(jt + 1) * 128, :], kstg)

                            for ib in range(IBLKS):
                                jtiles = 4 * (ib + 1)
                                width = 512 * (ib + 1)
                                PT = ptp.tile([128, ITILES, 512], BF16, tag="PT")
                                for il in range(4):
                                    it = ib * 4 + il
                                    P = prowp.tile([128, 4 * 512], BF16, tag="P")
                                    accs = []
                                    for jb in range(ib + 1):
                                        ps_s = pss.tile([128, 512], F32, tag="ps_s")
                                        nc.tensor.matmul(
                                            ps_s,
                                            qh[:, it * 128:(it + 1) * 128],
                                            kh[:, jb * 512:(jb + 1) * 512],
                                            start=True, stop=True)
                                        acc = statp.tile([128, 1], F32, tag="acc")
                                        if jb == ib:
                                            nc.vector.tensor_tensor(
                                                ps_s[:, il * 128:(il + 1) * 128],
                                                ps_s[:, il * 128:(il + 1) * 128],
                                                cmask, OP.add)
                                            wc = il * 128 + 128
                                            nc.scalar.activation(
                                                P[:, jb * 512: jb * 512 + wc],
                                                ps_s[:, :wc], AF.Exp,
                                                scale=SCALE, accum_out=acc)
                                            if wc < 512:
                                                nc.vector.memset(
                                                    P[:, jb * 512 + wc:(jb + 1) * 512], 0.0)
                                        else:
                                            nc.scalar.activation(
                                                P[:, jb * 512:(jb + 1) * 512],
                                                ps_s, AF.Exp,
                                                scale=SCALE, accum_out=acc)
                                        accs.append(acc)
                                    rs = accs[0]
                                    for a in accs[1:]:
                                        nc.vector.tensor_add(rs, rs, a)
                                    rinv = statp.tile([128, 1], F32, tag="rinv")
                                    nc.vector.reciprocal(rinv, rs)
                                    nc.vector.tensor_scalar_mul(
                                        P[:, :width], P[:, :width], rinv)
                                    for jt in range(jtiles):
                                        ptp_ps = pstr.tile([128, 128], BF16, tag="ptr")
                                        nc.tensor.transpose(
                                            ptp_ps, P[:, jt * 128:(jt + 1) * 128], ident)
                                        nc.any.tensor_copy(
                                            PT[:, jt, il * 128:(il + 1) * 128], ptp_ps)
                                ps_ct = psct.tile([128, 512], F32, tag="ps_ct")
                                for jt in range(jtiles):
                                    nc.tensor.matmul(
                                        ps_ct, v_sb[:, jt, :], PT[:, jt, :],
                                        start=(jt == 0), stop=(jt == jtiles - 1))
                                cstg = cstgp.tile([128, 512], BF16, tag="cstg")
                                nc.any.tensor_copy(cstg, ps_ct)
                                ctx_wr[b].append(nc.sync.dma_start(
                                    ctx_local[b][h * 128:(h + 1) * 128,
                                                 ib * 512:(ib + 1) * 512],
                                    cstg))

            # ---------------- stage 2.5: AllGather context ----------------
            ctx_ag = []
            for b in range(B):
                ag = nc.gpsimd.collective_compute(
                    "AllGather", OP.bypass,
                    replica_groups=[list(range(N_CORES))],
                    ins=[ctx_local[b].opt()],
                    outs=[ctx_all[b].opt()],
                )
                for wr in ctx_wr[b]:
                    _add_dep_helper(ag.ins, wr.ins, sync=True,
                                    reason="ctx allgather after ctx writes")
                ctx_ag.append(ag)

            # ---------------- stage 3: o_proj (column shard) ----------------
            with tc.tile_pool(name="wo3", bufs=1) as wo3, \
                 tc.tile_pool(name="cx3", bufs=3) as cx3, \
                 tc.tile_pool(name="ostg", bufs=3) as ostgp, \
                 tc.tile_pool(name="ps3", bufs=4, space="PSUM") as ps3:
                wo_sb = wo3.tile([128, KT, FPC], BF16)
                nc.sync.dma_start(wo_sb[:], wo.ap().rearrange("(ko p) n -> p ko n", p=128))
                for tt in range(T // 128):
                    b, ttl = divmod(tt, L // 128)
                    cx = cx3.tile([128, KT, 128], BF16, tag="cx")
                    for r in range(N_CORES):
                        cx_ld = nc.sync.dma_start(
                            cx[:, r * 4:(r + 1) * 4, :],
                            ctx_all[b][r].rearrange("(ks p) t -> p ks t", p=128)[
                                :, :, ttl * 128:(ttl + 1) * 128])
                        _add_dep_helper(cx_ld.ins, ctx_ag[b].ins, sync=True,
                                        reason="cx load after ctx allgather")
                    ps_o = ps3.tile([128, FPC], F32, tag="ps_o")
                    for kt in range(KT):
                        nc.tensor.matmul(
                            ps_o, cx[:, kt, :], wo_sb[:, kt, :],
                            start=(kt == 0), stop=(kt == KT - 1))
                    ostg = ostgp.tile([128, FPC], F32, tag="ostg")
                    nc.any.tensor_copy(ostg, ps_o)
                    nc.sync.dma_start(
                        out_cols.ap()[tt * 128:(tt + 1) * 128, :], ostg)

    n = split_excess_waits(nc, max_waits=1)
    print(f"split_excess_waits: inserted {n} NoOps")
    return nc


_RUNNER = None


def _get_nc():
    return _get_runner()["nc"]


def _get_runner():
    """Build the program once and a cached jitted SPMD executor around it.

    Mirrors bass2jax.run_bass_via_pjrt's multi-core path but (a) caches the
    jitted callable, (b) creates the donated output buffers on-device (no
    host->device transfer of zeros)."""
    global _RUNNER
    if _RUNNER is not None:
        return _RUNNER
    import jax
    import jax.numpy as jnp
    from jax.sharding import Mesh, PartitionSpec, NamedSharding
    try:
        from jax.experimental.shard_map import shard_map
    except ImportError:
        from jax import shard_map
    from concourse.bass2jax import (
        _bass_exec_p, install_neuronx_cc_hook, partition_id_tensor)

    nc = build_program()
    # Persistent compilation cache, keyed by BIR content (the default jax
    # cache key does not cover the custom-call payload, so a plain shared
    # cache dir returns stale executables when only the BIR changes).
    import hashlib
    import jax as _jax
    bir_hash = hashlib.sha1(nc.to_json_bytes()).hexdigest()[:16]
    try:
        _jax.config.update("jax_compilation_cache_dir",
                           f"/tmp/jax_kernel_cache_{bir_hash}")
        _jax.config.update("jax_persistent_cache_min_compile_time_secs", 0.0)
        _jax.config.update("jax_persistent_cache_min_entry_size_bytes", 0)
    except Exception:
        pass
    install_neuronx_cc_hook()

    in_names, out_names, out_avals = [], [], []
    partition_name = nc.partition_id_tensor.name if nc.partition_id_tensor else None
    for alloc in nc.m.functions[0].allocations:
        if not isinstance(alloc, mybir.MemoryLocationSet):
            continue
        name = alloc.memorylocations[0].name
        if alloc.kind == "ExternalInput":
            if name != partition_name:
                in_names.append(name)
        elif alloc.kind == "ExternalOutput":
            out_names.append(name)
            out_avals.append(jax.core.ShapedArray(
                tuple(alloc.tensor_shape), mybir.dt.np(alloc.dtype)))
    n_params = len(in_names)
    n_outs = len(out_avals)
    all_in_names = list(in_names) + list(out_names)
    if partition_name is not None:
        all_in_names.append(partition_name)

    def _body(*args):
        operands = list(args)
        if partition_name is not None:
            operands.append(partition_id_tensor())
        outs = _bass_exec_p.bind(
            *operands,
            out_avals=tuple(out_avals),
            in_names=tuple(all_in_names),
            out_names=tuple(out_names),
            lowering_input_output_aliases=(),
            sim_require_finite=True,
            sim_require_nnan=True,
            nc=nc,
        )
        return tuple(outs)

    devices = jax.devices()[:N_CORES]
    mesh = Mesh(np.asarray(devices), ("core",))
    in_specs = (PartitionSpec("core"),) * (n_params + n_outs)
    out_specs = (PartitionSpec("core"),) * n_outs
    donate = tuple(range(n_params, n_params + n_outs))
    sharded = jax.jit(
        shard_map(_body, mesh=mesh, in_specs=in_specs, out_specs=out_specs,
                  check_rep=False),
        donate_argnums=donate, keep_unused=True)

    zero_shardings = tuple(
        NamedSharding(mesh, PartitionSpec("core")) for _ in range(n_outs))
    zeros_fn = jax.jit(
        lambda: tuple(
            jnp.zeros((N_CORES * a.shape[0], *a.shape[1:]), a.dtype)
            for a in out_avals),
        out_shardings=zero_shardings)

    _RUNNER = dict(nc=nc, sharded=sharded, zeros_fn=zeros_fn,
                   in_names=in_names, out_names=out_names, out_avals=out_avals)
    return _RUNNER


def kernel(x, mask, Wq, bq, Wk, bk, Wv, bv, Wo, bo):
    r = _get_runner()
    bf = ml_dtypes.bfloat16
    xf = np.ascontiguousarray(np.asarray(x).reshape(T, D)).astype(bf)
    Wq = np.asarray(Wq); Wk = np.asarray(Wk); Wv = np.asarray(Wv); Wo = np.asarray(Wo)
    biases = {"bq": np.asarray(bq, dtype=np.float32),
              "bk": np.asarray(bk, dtype=np.float32),
              "bv": np.asarray(bv, dtype=np.float32)}
    ws = {"wq": Wq, "wk": Wk, "wv": Wv, "wo": Wo}
    # concatenated-over-cores (axis 0) global arrays, per input name
    concat_in = []
    for name in r["in_names"]:
        if name == "x_sh":
            concat_in.append(xf)  # [T, D] == concat of per-core [T/8, D]
        elif name in ws:
            w = ws[name]
            concat_in.append(np.concatenate(
                [np.ascontiguousarray(w[:, c * FPC:(c + 1) * FPC]).astype(bf)
                 for c in range(N_CORES)], axis=0))
        elif name in biases:
            concat_in.append(np.ascontiguousarray(biases[name]))  # [D] = concat of [FPC]
        else:
            raise KeyError(name)
    zeros = r["zeros_fn"]()
    out_arrs = r["sharded"](*concat_in, *zeros)
    res = {}
    for i, name in enumerate(r["out_names"]):
        a = np.asarray(out_arrs[i])
        res[name] = a.reshape(N_CORES, *r["out_avals"][i].shape)
    out = np.concatenate(list(res["out_cols"]), axis=1)
    out = out + np.asarray(bo, dtype=np.float32)[None, :]
    out = out.reshape(B, L, D).astype(np.float32)
    k = np.concatenate([res["k_out"][c].astype(np.float32)
                        for c in range(N_CORES)], axis=1)
    v = np.concatenate([res["v_out"][c].astype(np.float32)
                        for c in range(N_CORES)], axis=1)
    return out, k, v


# revision 17
# speedup vs baseline: 1.0145x; 1.0145x over previous
"""Multi-head causal attention on 8 Trainium2 NeuronCores.

Sharding: tensor-parallel over heads (4 heads/core) for QKV + attention;
AllGather of per-core context chunks (bf16, 4MB/core); o_proj column-sharded
(each core computes out[:, c*512:(c+1)*512] for all tokens). Final bias bo
and the output concatenation happen on host.

Compute dtype: bf16 operands, fp32 PSUM accumulation and softmax statistics.
Layouts: projections produce q/k/v feature-major ([feat, token]) so the
k-dim contraction runs on the partition axis everywhere; P tiles are
PE-transposed for the PV matmul; x is DMA-transposed on load (bf16 XBAR).
"""

import numpy as np
import ml_dtypes

import concourse.bass as bass
import concourse.mybir as mybir
import concourse.tile as tile
from concourse.bass_utils import run_bass_kernel_spmd
from concourse.masks import make_identity, make_causal_mask
from bass_rust import ScopedClock
from concourse.bass import _add_dep_helper

B, L, D = 2, 2048, 4096
H, HD = 32, 128
N_CORES = 8
HPC = H // N_CORES          # heads per core = 4
FPC = HPC * HD              # features per core = 512
T = B * L                   # tokens = 4096
SCALE = HD ** -0.5
MB = 512                    # stage-1 token block
BF16 = mybir.dt.bfloat16
F32 = mybir.dt.float32
AF = mybir.ActivationFunctionType
OP = mybir.AluOpType


class SplitDrainTileContext(tile.TileContext):
    """Tail drain in this walrus build allows only a couple of sync waits per
    instruction; split the global-clock waits across multiple drains."""

    MAX_WAITS = 1

    def _drain_and_barrier(self, tick_clock, wait_clock):
        nc = self.nc
        drain_inst = nc.sync.drain()
        wait_clock.add_sem_waits(
            drain_inst.ins, ScopedClock({None: tick_clock.global_clock})
        )
        si = drain_inst.ins.sync_info
        waits = list(si.on_wait) if si is not None and si.on_wait else []
        mw = self.MAX_WAITS
        if len(waits) > mw:
            si.on_wait = waits[:mw]
            rest = waits[mw:]
            for i in range(0, len(rest), mw):
                extra = nc.sync.drain()
                extra.ins.sync_info = mybir.SyncInfo(
                    on_wait=rest[i:i + mw], on_update=[]
                )
        nc.all_engine_barrier()
        assert self.sems is not None
        popped = nc._tile_sem_poison_stack.pop()
        assert popped is self._sem_poison
        nc.clear_and_free_semaphores(list(self.sems.allocated().values()))
        nc.all_engine_barrier()


def split_excess_waits(nc, max_waits=1):
    """walrus in this container rejects instructions with more than a couple of
    sync-wait commands. Split excess waits onto NoOps inserted just before the
    instruction on the same engine (same-engine program order preserved)."""
    f = nc.main_func
    new_blocks = []
    n = 0
    for bb in f.blocks:
        out = []
        changed = False
        for ins in bb.instructions:
            si = ins.sync_info
            waits = list(si.on_wait) if si is not None and si.on_wait else []
            if len(waits) > max_waits:
                keep, rest = waits[-max_waits:], waits[:-max_waits]
                for i in range(0, len(rest), max_waits):
                    nop = mybir.InstNoOp(name=f"I-wsplit-{n}", engine=ins.engine)
                    nop.sync_info = mybir.SyncInfo(
                        on_wait=rest[i:i + max_waits], on_update=[])
                    out.append(nop)
                    n += 1
                si.on_wait = keep
                changed = True
            out.append(ins)
        if changed:
            nb = mybir.BasicBlock(name=bb.name, instructions=out)
            for attr in ("IsExit", "IsLoopEntry", "IsPredicated"):
                v = getattr(bb, attr)
                if v is not None:
                    setattr(nb, attr, v)
            new_blocks.append(nb)
        else:
            new_blocks.append(bb)
    f.blocks = new_blocks
    return n


def build_program():
    nc = bass.Bass("TRN2", target_bir_lowering=False, debug=False,
                   num_devices=N_CORES)

    x_sh = nc.dram_tensor("x_sh", [T // N_CORES, D], BF16, kind="ExternalInput")
    wq = nc.dram_tensor("wq", [D, FPC], BF16, kind="ExternalInput")
    wk = nc.dram_tensor("wk", [D, FPC], BF16, kind="ExternalInput")
    wv = nc.dram_tensor("wv", [D, FPC], BF16, kind="ExternalInput")
    wo = nc.dram_tensor("wo", [D, FPC], BF16, kind="ExternalInput")
    bq = nc.dram_tensor("bq", [FPC], F32, kind="ExternalInput")
    bk = nc.dram_tensor("bk", [FPC], F32, kind="ExternalInput")
    bv = nc.dram_tensor("bv", [FPC], F32, kind="ExternalInput")

    k_out = nc.dram_tensor("k_out", [B, HPC, L, HD], BF16, kind="ExternalOutput")
    v_out = nc.dram_tensor("v_out", [B, HPC, L, HD], BF16, kind="ExternalOutput")
    out_cols = nc.dram_tensor("out_cols", [T, FPC], F32, kind="ExternalOutput")
    import os
    DBG = os.environ.get("KDBG") == "1"
    if DBG:
        dbg_xt = nc.dram_tensor("dbg_xt", [128, MB], BF16, kind="ExternalOutput")
        dbg_qt = nc.dram_tensor("dbg_qt", [FPC, T], BF16, kind="ExternalOutput")

    KT = D // 128            # 32 k-tiles
    MBLKS = T // MB          # stage-1 m blocks
    ITILES = L // 128        # 16 i-tiles per batch
    IBLKS = L // 512         # 4 i-blocks per batch

    with SplitDrainTileContext(nc) as tc:
        with tc.tile_pool(name="dram", bufs=1, space="DRAM") as dram:
            ctx_local = [dram.tile([FPC, L], BF16, name=f"ctx_local{b}")
                         for b in range(B)]
            ctx_all = [dram.tile([N_CORES, FPC, L], BF16, addr_space="Shared",
                                 name=f"ctx_all{b}") for b in range(B)]
            x_loc = dram.tile([T // N_CORES, D], BF16)
            x_all = dram.tile([N_CORES, T // N_CORES, D], BF16, addr_space="Shared")
            x_loc_wr = nc.sync.dma_start(x_loc[:], x_sh.ap()[:])
            x_ag = nc.gpsimd.collective_compute(
                "AllGather", OP.bypass,
                replica_groups=[list(range(N_CORES))],
                ins=[x_loc.opt()],
                outs=[x_all.opt()],
            )
            _add_dep_helper(x_ag.ins, x_loc_wr.ins, sync=True, reason="xag after xloc write")
            qT_d = dram.tile([FPC, T], BF16)
            kT_d = dram.tile([FPC, T], BF16)
            vT_d = dram.tile([FPC, T], BF16)

            if True:
                # ---------------- stage 1: QKV projections ----------------
                with tc.tile_pool(name="wres", bufs=1) as wres, \
                     tc.tile_pool(name="xt", bufs=2) as xtp, \
                     tc.tile_pool(name="stg1", bufs=4) as stg1, \
                     tc.tile_pool(name="ps1", bufs=4, space="PSUM") as ps1:
                    wq_sb = wres.tile([128, KT, FPC], BF16)
                    wk_sb = wres.tile([128, KT, FPC], BF16)
                    wv_sb = wres.tile([128, KT, FPC], BF16)
                    nc.sync.dma_start(wq_sb[:], wq.ap().rearrange("(ko p) n -> p ko n", p=128))
                    nc.sync.dma_start(wk_sb[:], wk.ap().rearrange("(ko p) n -> p ko n", p=128))
                    nc.sync.dma_start(wv_sb[:], wv.ap().rearrange("(ko p) n -> p ko n", p=128))
                    bq_sb = wres.tile([128, HPC], F32)
                    bk_sb = wres.tile([128, HPC], F32)
                    bv_sb = wres.tile([128, HPC], F32)
                    nc.sync.dma_start(bq_sb[:], bq.ap().rearrange("(o p) -> p o", p=128))
                    nc.sync.dma_start(bk_sb[:], bk.ap().rearrange("(o p) -> p o", p=128))
                    nc.sync.dma_start(bv_sb[:], bv.ap().rearrange("(o p) -> p o", p=128))

                    projs = [(wq_sb, bq_sb, qT_d), (wk_sb, bk_sb, kT_d), (wv_sb, bv_sb, vT_d)]
                    qkv_wr = {}
                    for mb in range(MBLKS):
                        xT = xtp.tile([128, KT, MB], BF16, tag="xT")
                        for kt in range(KT):
                            xt_tr = nc.sync.dma_start_transpose(
                                xT[:, kt, :],
                                x_all[mb, :, kt * 128:(kt + 1) * 128],
                            )
                            _add_dep_helper(xt_tr.ins, x_ag.ins, sync=True,
                                            reason="xT after x allgather")
                        if DBG and mb == 3:
                            nc.sync.dma_start(dbg_xt.ap()[:], xT[:, 2, :])
                        for pi, (w_sb, b_sb, oT) in enumerate(projs):
                            for nt in range(HPC):
                                ps = ps1.tile([128, MB], F32, tag="ps1")
                                for kt in range(KT):
                                    nc.tensor.matmul(
                                        ps, w_sb[:, kt, nt * 128:(nt + 1) * 128],
                                        xT[:, kt, :],
                                        start=(kt == 0), stop=(kt == KT - 1),
                                    )
                                stg = stg1.tile([128, MB], BF16, tag="stg1")
                                nc.scalar.activation(
                                    stg, ps,
                                    AF.Identity, bias=b_sb[:, nt:nt + 1],
                                )
                                qkv_wr[(pi, nt, mb)] = nc.sync.dma_start(
                                    oT[nt * 128:(nt + 1) * 128, mb * MB:(mb + 1) * MB],
                                    stg)

                if DBG:
                    nc.sync.dma_start(dbg_qt.ap()[:], qT_d[:])
                # ---------------- stage 2: attention ----------------
                with tc.tile_pool(name="const2", bufs=1) as cpool, \
                     tc.tile_pool(name="qkvh", bufs=3) as qkvhp, \
                     tc.tile_pool(name="kstg", bufs=4) as kstgp, \
                     tc.tile_pool(name="vtok", bufs=2) as vtokp, \
                     tc.tile_pool(name="prow", bufs=3) as prowp, \
                     tc.tile_pool(name="ptsl", bufs=2) as ptp, \
                     tc.tile_pool(name="stat", bufs=8) as statp, \
                     tc.tile_pool(name="cstg", bufs=4) as cstgp, \
                     tc.tile_pool(name="pstr", bufs=3, space="PSUM") as pstr, \
                     tc.tile_pool(name="pss", bufs=3, space="PSUM") as pss, \
                     tc.tile_pool(name="psct", bufs=2, space="PSUM") as psct:
                    ctx_wr = [[], []]
                    ident = cpool.tile([128, 128], BF16)
                    make_identity(nc, ident)
                    cmask = cpool.tile([128, 128], F32)
                    make_causal_mask(nc, cmask, mask_val=-1e10)

                    for b in range(B):
                        for h in range(HPC):
                            t0 = b * L
                            rows = slice(h * 128, (h + 1) * 128)
                            cols = slice(t0, t0 + L)
                            qh = qkvhp.tile([128, L], BF16, tag="qh")
                            kh = qkvhp.tile([128, L], BF16, tag="kh")
                            vh = qkvhp.tile([128, L], BF16, tag="vh")
                            mbs_b = range(4 * b, 4 * b + 4)
                            for pi, (ld_t, src_t) in enumerate(
                                    [(qh, qT_d), (kh, kT_d), (vh, vT_d)]):
                                ld = nc.sync.dma_start(ld_t[:], src_t[rows, cols])
                                for mb_ in mbs_b:
                                    _add_dep_helper(
                                        ld.ins, qkv_wr[(pi, h, mb_)].ins,
                                        sync=True, reason="qkv load after proj write")
                            # token-major v (matmul operand + output), k (output)
                            v_sb = vtokp.tile([128, ITILES, HD], BF16, tag="v_sb")
                            for jt in range(ITILES):
                                ptv = pstr.tile([128, 128], BF16, tag="ptr")
                                nc.tensor.transpose(
                                    ptv, vh[:, jt * 128:(jt + 1) * 128], ident)
                                nc.any.tensor_copy(v_sb[:, jt, :], ptv)
                                nc.sync.dma_start(
                                    v_out.ap()[b, h, jt * 128:(jt + 1) * 128, :],
                                    v_sb[:, jt, :])
                                ptk = pstr.tile([128, 128], BF16, tag="ptr")
                                nc.tensor.transpose(
                                    ptk, kh[:, jt * 128:(jt + 1) * 128], ident)
                                kstg = kstgp.tile([128, 128], BF16, tag="kstg")
                                nc.any.tensor_copy(kstg, ptk)
                                nc.sync.dma_start(
                                    k_out.ap()[b, h, jt * 128:(jt + 1) * 128, :], kstg)

                            for ib in range(IBLKS):
                                jtiles = 4 * (ib + 1)
                                width = 512 * (ib + 1)
                                PT = ptp.tile([128, ITILES, 512], BF16, tag="PT")
                                for il in range(4):
                                    it = ib * 4 + il
                                    P = prowp.tile([128, 4 * 512], BF16, tag="P")
                                    accs = []
                                    for jb in range(ib + 1):
                                        ps_s = pss.tile([128, 512], F32, tag="ps_s")
                                        nc.tensor.matmul(
                                            ps_s,
                                            qh[:, it * 128:(it + 1) * 128],
                                            kh[:, jb * 512:(jb + 1) * 512],
                                            start=True, stop=True)
                                        acc = statp.tile([128, 1], F32, tag="acc")
                                        if jb == ib:
                                            nc.vector.tensor_tensor(
                                                ps_s[:, il * 128:(il + 1) * 128],
                                                ps_s[:, il * 128:(il + 1) * 128],
                                                cmask, OP.add)
                                            wc = il * 128 + 128
                                            nc.scalar.activation(
                                                P[:, jb * 512: jb * 512 + wc],
                                                ps_s[:, :wc], AF.Exp,
                                                scale=SCALE, accum_out=acc)
                                            if wc < 512:
                                                nc.vector.memset(
                                                    P[:, jb * 512 + wc:(jb + 1) * 512], 0.0)
                                        else:
                                            nc.scalar.activation(
                                                P[:, jb * 512:(jb + 1) * 512],
                                                ps_s, AF.Exp,
                                                scale=SCALE, accum_out=acc)
                                        accs.append(acc)
                                    rs = accs[0]
                                    for a in accs[1:]:
                                        nc.vector.tensor_add(rs, rs, a)
                                    rinv = statp.tile([128, 1], F32, tag="rinv")
                                    nc.vector.reciprocal(rinv, rs)
                                    nc.vector.tensor_scalar_mul(
                                        P[:, :width], P[:, :width], rinv)
                                    for jt in range(jtiles):
                                        ptp_ps = pstr.tile([128, 128], BF16, tag="ptr")
                                        nc.tensor.transpose(
                                            ptp_ps, P[:, jt * 128:(jt + 1) * 128], ident)
                                        nc.any.tensor_copy(
                                            PT[:, jt, il * 128:(il + 1) * 128], ptp_ps)
                                ps_ct = psct.tile([128, 512], F32, tag="ps_ct")
                                for jt in range(jtiles):
                                    nc.tensor.matmul(
                                        ps_ct, v_sb[:, jt, :], PT[:, jt, :],
                                        start=(jt == 0), stop=(jt == jtiles - 1))
                                cstg = cstgp.tile([128, 512], BF16, tag="cstg")
                                nc.any.tensor_copy(cstg, ps_ct)
                                ctx_wr[b].append(nc.sync.dma_start(
                                    ctx_local[b][h * 128:(h + 1) * 128,
                                                 ib * 512:(ib + 1) * 512],
                                    cstg))

            # ---------------- stage 2.5: AllGather context ----------------
            ctx_ag = []
            for b in range(B):
                ag = nc.gpsimd.collective_compute(
                    "AllGather", OP.bypass,
                    replica_groups=[list(range(N_CORES))],
                    ins=[ctx_local[b].opt()],
                    outs=[ctx_all[b].opt()],
                )
                for wr in ctx_wr[b]:
                    _add_dep_helper(ag.ins, wr.ins, sync=True,
                                    reason="ctx allgather after ctx writes")
                ctx_ag.append(ag)

            # ---------------- stage 3: o_proj (column shard) ----------------
            with tc.tile_pool(name="wo3", bufs=1) as wo3, \
                 tc.tile_pool(name="cx3", bufs=3) as cx3, \
                 tc.tile_pool(name="ostg", bufs=3) as ostgp, \
                 tc.tile_pool(name="ps3", bufs=4, space="PSUM") as ps3:
                wo_sb = wo3.tile([128, KT, FPC], BF16)
                nc.sync.dma_start(wo_sb[:], wo.ap().rearrange("(ko p) n -> p ko n", p=128))
                for tt in range(T // 128):
                    b, ttl = divmod(tt, L // 128)
                    cx = cx3.tile([128, KT, 128], BF16, tag="cx")
                    for r in range(N_CORES):
                        cx_ld = nc.sync.dma_start(
                            cx[:, r * 4:(r + 1) * 4, :],
                            ctx_all[b][r].rearrange("(ks p) t -> p ks t", p=128)[
                                :, :, ttl * 128:(ttl + 1) * 128])
                        _add_dep_helper(cx_ld.ins, ctx_ag[b].ins, sync=True,
                                        reason="cx load after ctx allgather")
                    ps_o = ps3.tile([128, FPC], F32, tag="ps_o")
                    for kt in range(KT):
                        nc.tensor.matmul(
                            ps_o, cx[:, kt, :], wo_sb[:, kt, :],
                            start=(kt == 0), stop=(kt == KT - 1))
                    ostg = ostgp.tile([128, FPC], F32, tag="ostg")
                    nc.any.tensor_copy(ostg, ps_o)
                    nc.sync.dma_start(
                        out_cols.ap()[tt * 128:(tt + 1) * 128, :], ostg)

    n = split_excess_waits(nc, max_waits=1)
    print(f"split_excess_waits: inserted {n} NoOps")
    return nc


_RUNNER = None


def _get_nc():
    return _get_runner()["nc"]


def _get_runner():
    """Build the program once and a cached jitted SPMD executor around it.

    Mirrors bass2jax.run_bass_via_pjrt's multi-core path but (a) caches the
    jitted callable, (b) creates the donated output buffers on-device (no
    host->device transfer of zeros)."""
    global _RUNNER
    if _RUNNER is not None:
        return _RUNNER
    import jax
    import jax.numpy as jnp
    from jax.sharding import Mesh, PartitionSpec, NamedSharding
    try:
        from jax.experimental.shard_map import shard_map
    except ImportError:
        from jax import shard_map
    from concourse.bass2jax import (
        _bass_exec_p, install_neuronx_cc_hook, partition_id_tensor)

    nc = build_program()
    # Persistent compilation cache, keyed by BIR content (the default jax
    # cache key does not cover the custom-call payload, so a plain shared
    # cache dir returns stale executables when only the BIR changes).
    import jax as _jax
    try:
        _jax.config.update("jax_compilation_cache_dir",
                           "/tmp/jax_kernel_cache")
        _jax.config.update("jax_persistent_cache_min_compile_time_secs", 0.0)
        _jax.config.update("jax_persistent_cache_min_entry_size_bytes", 0)
    except Exception:
        pass
    install_neuronx_cc_hook()

    in_names, out_names, out_avals = [], [], []
    partition_name = nc.partition_id_tensor.name if nc.partition_id_tensor else None
    for alloc in nc.m.functions[0].allocations:
        if not isinstance(alloc, mybir.MemoryLocationSet):
            continue
        name = alloc.memorylocations[0].name
        if alloc.kind == "ExternalInput":
            if name != partition_name:
                in_names.append(name)
        elif alloc.kind == "ExternalOutput":
            out_names.append(name)
            out_avals.append(jax.core.ShapedArray(
                tuple(alloc.tensor_shape), mybir.dt.np(alloc.dtype)))
    n_params = len(in_names)
    n_outs = len(out_avals)
    all_in_names = list(in_names) + list(out_names)
    if partition_name is not None:
        all_in_names.append(partition_name)

    def _body(*args):
        operands = list(args)
        if partition_name is not None:
            operands.append(partition_id_tensor())
        outs = _bass_exec_p.bind(
            *operands,
            out_avals=tuple(out_avals),
            in_names=tuple(all_in_names),
            out_names=tuple(out_names),
            lowering_input_output_aliases=(),
            sim_require_finite=True,
            sim_require_nnan=True,
            nc=nc,
        )
        return tuple(outs)

    devices = jax.devices()[:N_CORES]
    mesh = Mesh(np.asarray(devices), ("core",))
    in_specs = (PartitionSpec("core"),) * (n_params + n_outs)
    out_specs = (PartitionSpec("core"),) * n_outs
    donate = tuple(range(n_params, n_params + n_outs))
    sharded = jax.jit(
        shard_map(_body, mesh=mesh, in_specs=in_specs, out_specs=out_specs,
                  check_rep=False),
        donate_argnums=donate, keep_unused=True)

    zero_shardings = tuple(
        NamedSharding(mesh, PartitionSpec("core")) for _ in range(n_outs))
    zeros_fn = jax.jit(
        lambda: tuple(
            jnp.zeros((N_CORES * a.shape[0], *a.shape[1:]), a.dtype)
            for a in out_avals),
        out_shardings=zero_shardings)

    _RUNNER = dict(nc=nc, sharded=sharded, zeros_fn=zeros_fn,
                   in_names=in_names, out_names=out_names, out_avals=out_avals)
    return _RUNNER


def kernel(x, mask, Wq, bq, Wk, bk, Wv, bv, Wo, bo):
    r = _get_runner()
    bf = ml_dtypes.bfloat16
    xf = np.ascontiguousarray(np.asarray(x).reshape(T, D)).astype(bf)
    Wq = np.asarray(Wq); Wk = np.asarray(Wk); Wv = np.asarray(Wv); Wo = np.asarray(Wo)
    biases = {"bq": np.asarray(bq, dtype=np.float32),
              "bk": np.asarray(bk, dtype=np.float32),
              "bv": np.asarray(bv, dtype=np.float32)}
    ws = {"wq": Wq, "wk": Wk, "wv": Wv, "wo": Wo}
    # concatenated-over-cores (axis 0) global arrays, per input name
    concat_in = []
    for name in r["in_names"]:
        if name == "x_sh":
            concat_in.append(xf)  # [T, D] == concat of per-core [T/8, D]
        elif name in ws:
            w = ws[name]
            concat_in.append(np.concatenate(
                [np.ascontiguousarray(w[:, c * FPC:(c + 1) * FPC]).astype(bf)
                 for c in range(N_CORES)], axis=0))
        elif name in biases:
            concat_in.append(np.ascontiguousarray(biases[name]))  # [D] = concat of [FPC]
        else:
            raise KeyError(name)
    zeros = r["zeros_fn"]()
    out_arrs = r["sharded"](*concat_in, *zeros)
    res = {}
    for i, name in enumerate(r["out_names"]):
        a = np.asarray(out_arrs[i])
        res[name] = a.reshape(N_CORES, *r["out_avals"][i].shape)
    out = np.concatenate(list(res["out_cols"]), axis=1)
    out = out + np.asarray(bo, dtype=np.float32)[None, :]
    out = out.reshape(B, L, D).astype(np.float32)
    k = np.concatenate([res["k_out"][c].astype(np.float32)
                        for c in range(N_CORES)], axis=1)
    v = np.concatenate([res["v_out"][c].astype(np.float32)
                        for c in range(N_CORES)], axis=1)
    return out, k, v


# revision 21
# speedup vs baseline: 1.0540x; 1.0390x over previous
"""Multi-head causal attention on 8 Trainium2 NeuronCores.

Sharding: tensor-parallel over heads (4 heads/core) for QKV + attention;
AllGather of per-core context chunks (bf16, 4MB/core); o_proj column-sharded
(each core computes out[:, c*512:(c+1)*512] for all tokens). Final bias bo
and the output concatenation happen on host.

Compute dtype: bf16 operands, fp32 PSUM accumulation and softmax statistics.
Layouts: projections produce q/k/v feature-major ([feat, token]) so the
k-dim contraction runs on the partition axis everywhere; P tiles are
PE-transposed for the PV matmul; x is DMA-transposed on load (bf16 XBAR).
"""

import numpy as np
import ml_dtypes

import concourse.bass as bass
import concourse.mybir as mybir
import concourse.tile as tile
from concourse.bass_utils import run_bass_kernel_spmd
from concourse.masks import make_identity, make_causal_mask
from bass_rust import ScopedClock
from concourse.bass import _add_dep_helper

B, L, D = 2, 2048, 4096
H, HD = 32, 128
N_CORES = 8
HPC = H // N_CORES          # heads per core = 4
FPC = HPC * HD              # features per core = 512
T = B * L                   # tokens = 4096
SCALE = HD ** -0.5
MB = 512                    # stage-1 token block
BF16 = mybir.dt.bfloat16
F32 = mybir.dt.float32
AF = mybir.ActivationFunctionType
OP = mybir.AluOpType


class SplitDrainTileContext(tile.TileContext):
    """Tail drain in this walrus build allows only a couple of sync waits per
    instruction; split the global-clock waits across multiple drains."""

    MAX_WAITS = 1

    def _drain_and_barrier(self, tick_clock, wait_clock):
        nc = self.nc
        drain_inst = nc.sync.drain()
        wait_clock.add_sem_waits(
            drain_inst.ins, ScopedClock({None: tick_clock.global_clock})
        )
        si = drain_inst.ins.sync_info
        waits = list(si.on_wait) if si is not None and si.on_wait else []
        mw = self.MAX_WAITS
        if len(waits) > mw:
            si.on_wait = waits[:mw]
            rest = waits[mw:]
            for i in range(0, len(rest), mw):
                extra = nc.sync.drain()
                extra.ins.sync_info = mybir.SyncInfo(
                    on_wait=rest[i:i + mw], on_update=[]
                )
        nc.all_engine_barrier()
        assert self.sems is not None
        popped = nc._tile_sem_poison_stack.pop()
        assert popped is self._sem_poison
        nc.clear_and_free_semaphores(list(self.sems.allocated().values()))
        nc.all_engine_barrier()


def split_excess_waits(nc, max_waits=1):
    """walrus in this container rejects instructions with more than a couple of
    sync-wait commands. Split excess waits onto NoOps inserted just before the
    instruction on the same engine (same-engine program order preserved)."""
    f = nc.main_func
    new_blocks = []
    n = 0
    for bb in f.blocks:
        out = []
        changed = False
        for ins in bb.instructions:
            si = ins.sync_info
            waits = list(si.on_wait) if si is not None and si.on_wait else []
            if len(waits) > max_waits:
                keep, rest = waits[-max_waits:], waits[:-max_waits]
                for i in range(0, len(rest), max_waits):
                    nop = mybir.InstNoOp(name=f"I-wsplit-{n}", engine=ins.engine)
                    nop.sync_info = mybir.SyncInfo(
                        on_wait=rest[i:i + max_waits], on_update=[])
                    out.append(nop)
                    n += 1
                si.on_wait = keep
                changed = True
            out.append(ins)
        if changed:
            nb = mybir.BasicBlock(name=bb.name, instructions=out)
            for attr in ("IsExit", "IsLoopEntry", "IsPredicated"):
                v = getattr(bb, attr)
                if v is not None:
                    setattr(nb, attr, v)
            new_blocks.append(nb)
        else:
            new_blocks.append(bb)
    f.blocks = new_blocks
    return n


def build_program():
    nc = bass.Bass("TRN2", target_bir_lowering=False, debug=False,
                   num_devices=N_CORES)

    x_sh = nc.dram_tensor("x_sh", [T // N_CORES, D], BF16, kind="ExternalInput")
    wq = nc.dram_tensor("wq", [D, FPC], BF16, kind="ExternalInput")
    wk = nc.dram_tensor("wk", [D, FPC], BF16, kind="ExternalInput")
    wv = nc.dram_tensor("wv", [D, FPC], BF16, kind="ExternalInput")
    wo = nc.dram_tensor("wo", [D, FPC], BF16, kind="ExternalInput")
    bq = nc.dram_tensor("bq", [FPC], F32, kind="ExternalInput")
    bk = nc.dram_tensor("bk", [FPC], F32, kind="ExternalInput")
    bv = nc.dram_tensor("bv", [FPC], F32, kind="ExternalInput")

    k_out = nc.dram_tensor("k_out", [B, HPC, L, HD], BF16, kind="ExternalOutput")
    v_out = nc.dram_tensor("v_out", [B, HPC, L, HD], BF16, kind="ExternalOutput")
    out_cols = nc.dram_tensor("out_cols", [T, FPC], F32, kind="ExternalOutput")
    import os
    DBG = os.environ.get("KDBG") == "1"
    if DBG:
        dbg_xt = nc.dram_tensor("dbg_xt", [128, MB], BF16, kind="ExternalOutput")
        dbg_qt = nc.dram_tensor("dbg_qt", [FPC, T], BF16, kind="ExternalOutput")

    KT = D // 128            # 32 k-tiles
    MBLKS = T // MB          # stage-1 m blocks
    ITILES = L // 128        # 16 i-tiles per batch
    IBLKS = L // 512         # 4 i-blocks per batch

    with SplitDrainTileContext(nc) as tc:
        with tc.tile_pool(name="dram", bufs=1, space="DRAM") as dram:
            ctx_local = [dram.tile([FPC, L], BF16, name=f"ctx_local{b}")
                         for b in range(B)]
            ctx_all = [dram.tile([N_CORES, FPC, L], BF16, addr_space="Shared",
                                 name=f"ctx_all{b}") for b in range(B)]
            x_loc = dram.tile([T // N_CORES, D], BF16)
            x_all = dram.tile([N_CORES, T // N_CORES, D], BF16, addr_space="Shared")
            x_loc_wr = nc.sync.dma_start(x_loc[:], x_sh.ap()[:])
            x_ag = nc.gpsimd.collective_compute(
                "AllGather", OP.bypass,
                replica_groups=[list(range(N_CORES))],
                ins=[x_loc.opt()],
                outs=[x_all.opt()],
            )
            _add_dep_helper(x_ag.ins, x_loc_wr.ins, sync=True, reason="xag after xloc write")
            qT_d = dram.tile([FPC, T], BF16)
            kT_d = dram.tile([FPC, T], BF16)
            vT_d = dram.tile([FPC, T], BF16)

            if True:
                # ---------------- stage 1: QKV projections ----------------
                with tc.tile_pool(name="wres", bufs=1) as wres, \
                     tc.tile_pool(name="xt", bufs=2) as xtp, \
                     tc.tile_pool(name="stg1", bufs=4) as stg1, \
                     tc.tile_pool(name="ps1", bufs=4, space="PSUM") as ps1:
                    wq_sb = wres.tile([128, KT, FPC], BF16)
                    wk_sb = wres.tile([128, KT, FPC], BF16)
                    wv_sb = wres.tile([128, KT, FPC], BF16)
                    nc.sync.dma_start(wq_sb[:], wq.ap().rearrange("(ko p) n -> p ko n", p=128))
                    nc.sync.dma_start(wk_sb[:], wk.ap().rearrange("(ko p) n -> p ko n", p=128))
                    nc.sync.dma_start(wv_sb[:], wv.ap().rearrange("(ko p) n -> p ko n", p=128))
                    bq_sb = wres.tile([128, HPC], F32)
                    bk_sb = wres.tile([128, HPC], F32)
                    bv_sb = wres.tile([128, HPC], F32)
                    nc.sync.dma_start(bq_sb[:], bq.ap().rearrange("(o p) -> p o", p=128))
                    nc.sync.dma_start(bk_sb[:], bk.ap().rearrange("(o p) -> p o", p=128))
                    nc.sync.dma_start(bv_sb[:], bv.ap().rearrange("(o p) -> p o", p=128))

                    projs = [(wq_sb, bq_sb, qT_d), (wk_sb, bk_sb, kT_d), (wv_sb, bv_sb, vT_d)]
                    qkv_wr = {}
                    for mb in range(MBLKS):
                        xT = xtp.tile([128, KT, MB], BF16, tag="xT")
                        for kt in range(KT):
                            xt_tr = nc.sync.dma_start_transpose(
                                xT[:, kt, :],
                                x_all[mb, :, kt * 128:(kt + 1) * 128],
                            )
                            _add_dep_helper(xt_tr.ins, x_ag.ins, sync=True,
                                            reason="xT after x allgather")
                        if DBG and mb == 3:
                            nc.sync.dma_start(dbg_xt.ap()[:], xT[:, 2, :])
                        for pi, (w_sb, b_sb, oT) in enumerate(projs):
                            for nt in range(HPC):
                                ps = ps1.tile([128, MB], F32, tag="ps1")
                                for kt in range(KT):
                                    nc.tensor.matmul(
                                        ps, w_sb[:, kt, nt * 128:(nt + 1) * 128],
                                        xT[:, kt, :],
                                        start=(kt == 0), stop=(kt == KT - 1),
                                    )
                                stg = stg1.tile([128, MB], BF16, tag="stg1")
                                nc.scalar.activation(
                                    stg, ps,
                                    AF.Identity, bias=b_sb[:, nt:nt + 1],
                                )
                                qkv_wr[(pi, nt, mb)] = nc.sync.dma_start(
                                    oT[nt * 128:(nt + 1) * 128, mb * MB:(mb + 1) * MB],
                                    stg)

                if DBG:
                    nc.sync.dma_start(dbg_qt.ap()[:], qT_d[:])
                # ---------------- stage 2: attention ----------------
                with tc.tile_pool(name="const2", bufs=1) as cpool, \
                     tc.tile_pool(name="qkvh", bufs=3) as qkvhp, \
                     tc.tile_pool(name="kstg", bufs=4) as kstgp, \
                     tc.tile_pool(name="vtok", bufs=2) as vtokp, \
                     tc.tile_pool(name="prow", bufs=3) as prowp, \
                     tc.tile_pool(name="ptsl", bufs=2) as ptp, \
                     tc.tile_pool(name="stat", bufs=8) as statp, \
                     tc.tile_pool(name="cstg", bufs=4) as cstgp, \
                     tc.tile_pool(name="pstr", bufs=3, space="PSUM") as pstr, \
                     tc.tile_pool(name="pss", bufs=3, space="PSUM") as pss, \
                     tc.tile_pool(name="psct", bufs=2, space="PSUM") as psct:
                    ctx_wr = [[], []]
                    ident = cpool.tile([128, 128], BF16)
                    make_identity(nc, ident)
                    cmask = cpool.tile([128, 128], F32)
                    make_causal_mask(nc, cmask, mask_val=-1e10)

                    for b in range(B):
                        for h in range(HPC):
                            t0 = b * L
                            rows = slice(h * 128, (h + 1) * 128)
                            cols = slice(t0, t0 + L)
                            qh = qkvhp.tile([128, L], BF16, tag="qh")
                            kh = qkvhp.tile([128, L], BF16, tag="kh")
                            vh = qkvhp.tile([128, L], BF16, tag="vh")
                            mbs_b = range(4 * b, 4 * b + 4)
                            for pi, (ld_t, src_t) in enumerate(
                                    [(qh, qT_d), (kh, kT_d), (vh, vT_d)]):
                                ld = nc.sync.dma_start(ld_t[:], src_t[rows, cols])
                                for mb_ in mbs_b:
                                    _add_dep_helper(
                                        ld.ins, qkv_wr[(pi, h, mb_)].ins,
                                        sync=True, reason="qkv load after proj write")
                            # token-major v (matmul operand + output), k (output)
                            v_sb = vtokp.tile([128, ITILES, HD], BF16, tag="v_sb")
                            for jt in range(ITILES):
                                ptv = pstr.tile([128, 128], BF16, tag="ptr")
                                nc.tensor.transpose(
                                    ptv, vh[:, jt * 128:(jt + 1) * 128], ident)
                                nc.any.tensor_copy(v_sb[:, jt, :], ptv)
                                nc.sync.dma_start(
                                    v_out.ap()[b, h, jt * 128:(jt + 1) * 128, :],
                                    v_sb[:, jt, :])
                                ptk = pstr.tile([128, 128], BF16, tag="ptr")
                                nc.tensor.transpose(
                                    ptk, kh[:, jt * 128:(jt + 1) * 128], ident)
                                kstg = kstgp.tile([128, 128], BF16, tag="kstg")
                                nc.any.tensor_copy(kstg, ptk)
                                nc.sync.dma_start(
                                    k_out.ap()[b, h, jt * 128:(jt + 1) * 128, :], kstg)

                            for ib in range(IBLKS):
                                jtiles = 4 * (ib + 1)
                                width = 512 * (ib + 1)
                                PT = ptp.tile([128, ITILES, 512], BF16, tag="PT")
                                for il in range(4):
                                    it = ib * 4 + il
                                    P = prowp.tile([128, 4 * 512], BF16, tag="P")
                                    accs = []
                                    for jb in range(ib + 1):
                                        ps_s = pss.tile([128, 512], F32, tag="ps_s")
                                        nc.tensor.matmul(
                                            ps_s,
                                            qh[:, it * 128:(it + 1) * 128],
                                            kh[:, jb * 512:(jb + 1) * 512],
                                            start=True, stop=True)
                                        acc = statp.tile([128, 1], F32, tag="acc")
                                        if jb == ib:
                                            nc.vector.tensor_tensor(
                                                ps_s[:, il * 128:(il + 1) * 128],
                                                ps_s[:, il * 128:(il + 1) * 128],
                                                cmask, OP.add)
                                            wc = il * 128 + 128
                                            nc.scalar.activation(
                                                P[:, jb * 512: jb * 512 + wc],
                                                ps_s[:, :wc], AF.Exp,
                                                scale=SCALE, accum_out=acc)
                                            if wc < 512:
                                                nc.vector.memset(
                                                    P[:, jb * 512 + wc:(jb + 1) * 512], 0.0)
                                        else:
                                            nc.scalar.activation(
                                                P[:, jb * 512:(jb + 1) * 512],
                                                ps_s, AF.Exp,
                                                scale=SCALE, accum_out=acc)
                                        accs.append(acc)
                                    rs = accs[0]
                                    for a in accs[1:]:
                                        nc.vector.tensor_add(rs, rs, a)
                                    rinv = statp.tile([128, 1], F32, tag="rinv")
                                    nc.vector.reciprocal(rinv, rs)
                                    nc.vector.tensor_scalar_mul(
                                        P[:, :width], P[:, :width], rinv)
                                    for jt in range(jtiles):
                                        ptp_ps = pstr.tile([128, 128], BF16, tag="ptr")
                                        nc.tensor.transpose(
                                            ptp_ps, P[:, jt * 128:(jt + 1) * 128], ident)
                                        nc.any.tensor_copy(
                                            PT[:, jt, il * 128:(il + 1) * 128], ptp_ps)
                                ps_ct = psct.tile([128, 512], F32, tag="ps_ct")
                                for jt in range(jtiles):
                                    nc.tensor.matmul(
                                        ps_ct, v_sb[:, jt, :], PT[:, jt, :],
                                        start=(jt == 0), stop=(jt == jtiles - 1))
                                cstg = cstgp.tile([128, 512], BF16, tag="cstg")
                                nc.any.tensor_copy(cstg, ps_ct)
                                ctx_wr[b].append(nc.sync.dma_start(
                                    ctx_local[b][h * 128:(h + 1) * 128,
                                                 ib * 512:(ib + 1) * 512],
                                    cstg))

            # ---------------- stage 2.5: AllGather context ----------------
            ctx_ag = []
            for b in range(B):
                ag = nc.gpsimd.collective_compute(
                    "AllGather", OP.bypass,
                    replica_groups=[list(range(N_CORES))],
                    ins=[ctx_local[b].opt()],
                    outs=[ctx_all[b].opt()],
                )
                for wr in ctx_wr[b]:
                    _add_dep_helper(ag.ins, wr.ins, sync=True,
                                    reason="ctx allgather after ctx writes")
                ctx_ag.append(ag)

            # ---------------- stage 3: o_proj (column shard) ----------------
            with tc.tile_pool(name="wo3", bufs=1) as wo3, \
                 tc.tile_pool(name="cx3", bufs=3) as cx3, \
                 tc.tile_pool(name="ostg", bufs=3) as ostgp, \
                 tc.tile_pool(name="ps3", bufs=4, space="PSUM") as ps3:
                wo_sb = wo3.tile([128, KT, FPC], BF16)
                nc.sync.dma_start(wo_sb[:], wo.ap().rearrange("(ko p) n -> p ko n", p=128))
                for tt in range(T // 128):
                    b, ttl = divmod(tt, L // 128)
                    cx = cx3.tile([128, KT, 128], BF16, tag="cx")
                    for r in range(N_CORES):
                        cx_ld = nc.sync.dma_start(
                            cx[:, r * 4:(r + 1) * 4, :],
                            ctx_all[b][r].rearrange("(ks p) t -> p ks t", p=128)[
                                :, :, ttl * 128:(ttl + 1) * 128])
                        _add_dep_helper(cx_ld.ins, ctx_ag[b].ins, sync=True,
                                        reason="cx load after ctx allgather")
                    ps_o = ps3.tile([128, FPC], F32, tag="ps_o")
                    for kt in range(KT):
                        nc.tensor.matmul(
                            ps_o, cx[:, kt, :], wo_sb[:, kt, :],
                            start=(kt == 0), stop=(kt == KT - 1))
                    ostg = ostgp.tile([128, FPC], F32, tag="ostg")
                    nc.any.tensor_copy(ostg, ps_o)
                    nc.sync.dma_start(
                        out_cols.ap()[tt * 128:(tt + 1) * 128, :], ostg)

    n = split_excess_waits(nc, max_waits=1)
    print(f"split_excess_waits: inserted {n} NoOps")
    return nc


_RUNNER = None


def _get_nc():
    return _get_runner()["nc"]


def _get_runner():
    """Build the program once and a cached jitted SPMD executor around it.

    Mirrors bass2jax.run_bass_via_pjrt's multi-core path but (a) caches the
    jitted callable, (b) creates the donated output buffers on-device (no
    host->device transfer of zeros)."""
    global _RUNNER
    if _RUNNER is not None:
        return _RUNNER
    import jax
    import jax.numpy as jnp
    from jax.sharding import Mesh, PartitionSpec, NamedSharding
    try:
        from jax.experimental.shard_map import shard_map
    except ImportError:
        from jax import shard_map
    from concourse.bass2jax import (
        _bass_exec_p, install_neuronx_cc_hook, partition_id_tensor)

    nc = build_program()
    # Persistent compilation cache, keyed by BIR content (the default jax
    # cache key does not cover the custom-call payload, so a plain shared
    # cache dir returns stale executables when only the BIR changes).
    import jax as _jax
    try:
        _jax.config.update("jax_compilation_cache_dir",
                           "/tmp/jax_kernel_cache")
        _jax.config.update("jax_persistent_cache_min_compile_time_secs", 0.0)
        _jax.config.update("jax_persistent_cache_min_entry_size_bytes", 0)
    except Exception:
        pass
    install_neuronx_cc_hook()

    in_names, out_names, out_avals = [], [], []
    partition_name = nc.partition_id_tensor.name if nc.partition_id_tensor else None
    for alloc in nc.m.functions[0].allocations:
        if not isinstance(alloc, mybir.MemoryLocationSet):
            continue
        name = alloc.memorylocations[0].name
        if alloc.kind == "ExternalInput":
            if name != partition_name:
                in_names.append(name)
        elif alloc.kind == "ExternalOutput":
            out_names.append(name)
            out_avals.append(jax.core.ShapedArray(
                tuple(alloc.tensor_shape), mybir.dt.np(alloc.dtype)))
    n_params = len(in_names)
    n_outs = len(out_avals)
    all_in_names = list(in_names) + list(out_names)
    if partition_name is not None:
        all_in_names.append(partition_name)

    def _body(*args):
        operands = list(args)
        if partition_name is not None:
            operands.append(partition_id_tensor())
        outs = _bass_exec_p.bind(
            *operands,
            out_avals=tuple(out_avals),
            in_names=tuple(all_in_names),
            out_names=tuple(out_names),
            lowering_input_output_aliases=(),
            sim_require_finite=True,
            sim_require_nnan=True,
            nc=nc,
        )
        return tuple(outs)

    devices = jax.devices()[:N_CORES]
    mesh = Mesh(np.asarray(devices), ("core",))
    in_specs = (PartitionSpec("core"),) * (n_params + n_outs)
    out_specs = (PartitionSpec("core"),) * n_outs
    donate = tuple(range(n_params, n_params + n_outs))
    sharded = jax.jit(
        shard_map(_body, mesh=mesh, in_specs=in_specs, out_specs=out_specs,
                  check_rep=False),
        donate_argnums=donate, keep_unused=True)

    zero_shardings = tuple(
        NamedSharding(mesh, PartitionSpec("core")) for _ in range(n_outs))
    zeros_fn = jax.jit(
        lambda: tuple(
            jnp.zeros((N_CORES * a.shape[0], *a.shape[1:]), a.dtype)
            for a in out_avals),
        out_shardings=zero_shardings)

    _RUNNER = dict(nc=nc, sharded=sharded, zeros_fn=zeros_fn,
                   in_names=in_names, out_names=out_names, out_avals=out_avals)
    return _RUNNER


def kernel(x, mask, Wq, bq, Wk, bk, Wv, bv, Wo, bo):
    r = _get_runner()
    bf = ml_dtypes.bfloat16
    xf = np.ascontiguousarray(np.asarray(x).reshape(T, D)).astype(bf)
    Wq = np.asarray(Wq); Wk = np.asarray(Wk); Wv = np.asarray(Wv); Wo = np.asarray(Wo)
    biases = {"bq": np.asarray(bq, dtype=np.float32),
              "bk": np.asarray(bk, dtype=np.float32),
              "bv": np.asarray(bv, dtype=np.float32)}
    ws = {"wq": Wq, "wk": Wk, "wv": Wv, "wo": Wo}
    # concatenated-over-cores (axis 0) global arrays, per input name
    concat_in = []
    for name in r["in_names"]:
        if name == "x_sh":
            concat_in.append(xf)  # [T, D] == concat of per-core [T/8, D]
        elif name in ws:
            w = ws[name]
            concat_in.append(np.concatenate(
                [np.ascontiguousarray(w[:, c * FPC:(c + 1) * FPC]).astype(bf)
                 for c in range(N_CORES)], axis=0))
        elif name in biases:
            concat_in.append(np.ascontiguousarray(biases[name]))  # [D] = concat of [FPC]
        else:
            raise KeyError(name)
    zeros = r["zeros_fn"]()
    out_arrs = r["sharded"](*concat_in, *zeros)
    res = {}
    for i, name in enumerate(r["out_names"]):
        a = np.asarray(out_arrs[i])
        res[name] = a.reshape(N_CORES, *r["out_avals"][i].shape)
    out = np.concatenate(list(res["out_cols"]), axis=1)
    out = out + np.asarray(bo, dtype=np.float32)[None, :]
    out = out.reshape(B, L, D).astype(np.float32)
    k = np.concatenate([res["k_out"][c].astype(np.float32)
                        for c in range(N_CORES)], axis=1)
    v = np.concatenate([res["v_out"][c].astype(np.float32)
                        for c in range(N_CORES)], axis=1)
    return out, k, v
